# revision 40
# baseline (speedup 1.0000x reference)
"""2-layer GCN (GEMM -> COO SpMM -> ReLU -> GEMM -> SpMM) on 8 trn2 NeuronCores.

v2 design (one collective, X-space layer 1):
  - A@(X W1 + b1) = (A@X) W1 + deg.b1^T with deg = A@1 (host bincount).
    X is a full input replicated on every core, so layer 1 needs NO
    collective: gather X rows directly (256 bf16 feats = 512B descriptors,
    full DMA efficiency), accumulate (A@X)^T per 128-row dest block on PE
    via edge-slot scatter matmuls, then a small local GEMM + rank-1 bias.
  - relu1 [RPC rows, 128] bf16 is written row-major and AllGather'd once
    (Shared-output HBM collective = NRT fast path).
  - Layer 2 = baseline SpMM structure over relu1_full: gather by (block,
    quartile), scatter-matmul to (A relu1)^T, local GEMM2 + rank-1 bias,
    row-major f32 output (no host transpose), out-writes batched per group.
  - dma_gather instructions stay <= 1024 descriptors with the default 16KB
    SWDGE scratch: larger rings/scratch sizes hang the device.
  - Block GEMMs are deferred by one group (consume closures) so psum
    evacuation never bubbles the PE; evacs split across Act and DVE.

PSUM budget (8 banks): scatter pool 6 (L1: 3 blocks x 2 halves, L2:
6 blocks x 1) + gemm pool 2.

SPMD: one program for 8 cores; per-core data varies only in input tensors.
Slot layout per (block, quartile) padded to CAP chunks of 128; pad slots
gather row 0 with val 0 (harmless).
"""

import sys

import numpy as np
import ml_dtypes

_TRN_REPO = "/opt/trn_rl_repo"
if _TRN_REPO not in sys.path:
    sys.path.insert(0, _TRN_REPO)

import concourse.bass as bass
import concourse.tile as tile
from concourse import bacc, mybir
from concourse.bass_utils import run_bass_kernel_spmd

BF16 = mybir.dt.bfloat16
F32 = mybir.dt.float32
I16 = mybir.dt.int16

RING = 1024          # SWDGE descriptor ring (scratch 16384 / 16)
SCRATCH = 16384


class Cfg:
    def __init__(self, n_nodes, in_size, hidden, out_size):
        self.M = 8
        self.NN = n_nodes
        self.IN = in_size
        self.HID = hidden
        self.OUT = out_size
        assert n_nodes % self.M == 0
        self.RPC = n_nodes // self.M          # real rows per core
        self.BL = 128
        self.NB = (self.RPC + 127) // 128
        self.RPAD = self.NB * 128             # padded rows per core
        self.NNP = self.M * self.RPAD         # padded global nodes
        self.Q = 4
        self.QB1 = (n_nodes + self.Q - 1) // self.Q      # X-space quartile
        self.QB2 = self.NNP // self.Q                    # padded-row quartile
        assert self.QB1 <= 32768 and self.QB2 <= 32768
        self.GB1 = 3                          # blocks/group, layer 1 (2 psum each)
        self.GB2 = 6                          # blocks/group, layer 2 (1 psum each)
        assert in_size % 128 == 0 and hidden == 128 and out_size == 128


FULL = Cfg(100000, 256, 128, 128)


def _plan_layer(cfg, er, ec, ev, qbase, gb):
    """Slot layout for one SpMM layer: edges (er=dest row local, ec=source
    index, ev=val) per core, bucketed by (dest block, source quartile).

    Fine-grained packing: each (b, q) segment gets a SHARED capacity =
    max-over-cores rounded up to 16 (idx alignment); segments concatenate
    within a (group, quartile) run (padded to x128 for chunk alignment), so
    chunks of 128 slots can span block boundaries.  Each (chunk, touched
    block) pair gets its own (rloc, sval) fpack column: slots outside the
    block get rloc=200 (never equal to iota 0..127) and sval=0.
    """
    M = len(er)
    NB = cfg.NB
    nq = cfg.Q
    # shared per-(b, q) capacities
    cnt = np.zeros((M, NB * nq), dtype=np.int64)
    for m in range(M):
        key = (er[m] // cfg.BL) * nq + np.minimum(ec[m] // qbase, nq - 1)
        cnt[m] = np.bincount(key, minlength=NB * nq)
    cap16 = ((cnt.max(axis=0) + 15) // 16 * 16).reshape(NB, nq)

    groups = [list(range(g, min(g + gb, NB))) for g in range(0, NB, gb)]
    slot_off = {}
    runs = []   # per (group, quartile): dict(q, ioff, n, insts)
    off = 0
    ninst = 0
    for blist in groups:
        for q in range(nq):
            ioff = off
            for b in blist:
                slot_off[(b, q)] = off
                off += int(cap16[b, q])
            n = -(-(off - ioff) // 128) * 128          # pad run to x128
            off = ioff + n
            # chunk -> touched blocks
            insts = []
            for c in range(n // 128):
                lo, hi = ioff + c * 128, ioff + (c + 1) * 128
                touched = [b for b in blist
                           if slot_off[(b, q)] < hi and
                           slot_off[(b, q)] + int(cap16[b, q]) > lo]
                ilist = []
                for b in touched:
                    ilist.append([blist.index(b), b, ninst, False, False])
                    ninst += 1
                insts.append(ilist)
            runs.append(dict(q=q, ioff=ioff, n=n, insts=insts))
    nslot = off
    nchunk = nslot // 128
    nfcol = ninst

    # start/stop flags: first/last instance per block in emission order
    first_seen = {}
    last_seen = {}
    for run in runs:
        for ilist in run["insts"]:
            for inst in ilist:
                b = inst[1]
                if b not in first_seen:
                    first_seen[b] = inst
                last_seen[b] = inst
    for b, inst in first_seen.items():
        inst[3] = True
    for b, inst in last_seen.items():
        inst[4] = True

    # shared owner map: slot -> owning block (-1 for run pad)
    owner = np.full(nslot, -1, dtype=np.int64)
    for (b, q), so in slot_off.items():
        owner[so:so + int(cap16[b, q])] = b

    per_core = []
    for m in range(M):
        erm, ecm, evm = er[m], ec[m], ev[m]
        blk = erm // cfg.BL
        order = np.lexsort((ecm, blk))
        erm, ecm, evm, blk = erm[order], ecm[order], evm[order], blk[order]

        idx16 = np.zeros(nslot, dtype=np.int16)
        rloc = np.full(nslot, 200.0, dtype=np.float32)
        sval = np.zeros(nslot, dtype=np.float32)

        bstart = np.searchsorted(blk, np.arange(NB + 1))
        for b in range(NB):
            i0, i1 = bstart[b], bstart[b + 1]
            ecb = ecm[i0:i1]
            qsplit = np.searchsorted(ecb, np.arange(nq + 1) * qbase)
            for q in range(nq):
                j0, j1 = i0 + qsplit[q], i0 + qsplit[q + 1]
                n = j1 - j0
                assert n <= cap16[b, q]
                so = slot_off[(b, q)]
                idx16[so:so + n] = (ecm[j0:j1] - q * qbase).astype(np.int16)
                rloc[so:so + n] = (erm[j0:j1] - b * cfg.BL).astype(np.float32)
                sval[so:so + n] = evm[j0:j1]

        # per-instance fpack columns [128, nfcol]
        rcols = np.full((128, nfcol), 200.0, dtype=np.float32)
        scols = np.zeros((128, nfcol), dtype=np.float32)
        for run in runs:
            for c, ilist in enumerate(run["insts"]):
                lo = run["ioff"] + c * 128
                wo = owner[lo:lo + 128]
                wr = rloc[lo:lo + 128]
                ws = sval[lo:lo + 128]
                for bi, b, col, _, _ in ilist:
                    sel = wo == b
                    rcols[:, col] = np.where(sel, wr, 200.0)
                    scols[:, col] = np.where(sel, ws, 0.0)

        idx_w = np.tile(idx16.reshape(-1, 16).T, (8, 1))
        per_core.append(dict(
            idx=np.ascontiguousarray(idx_w),
            rloc=np.ascontiguousarray(rcols),
            sval=np.ascontiguousarray(scols)))
    return dict(groups=groups, runs=runs, nslot=nslot, nchunk=nchunk,
                nfcol=nfcol, per_core=per_core)


def build_plan(cfg, row, col, vals):
    row = np.asarray(row).astype(np.int64)
    col = np.asarray(col).astype(np.int64)
    vals = np.asarray(vals).astype(np.float32)

    er1, ec1, ev1 = [], [], []
    er2, ec2, ev2 = [], [], []
    colp = (col // cfg.RPC) * cfg.RPAD + (col % cfg.RPC)  # padded-row space
    for m in range(cfg.M):
        sel = (row // cfg.RPC) == m
        er = (row[sel] - m * cfg.RPC).astype(np.int64)
        ev = vals[sel]
        er1.append(er); ec1.append(col[sel]); ev1.append(ev)
        er2.append(er.copy()); ec2.append(colp[sel]); ev2.append(ev.copy())

    L1 = _plan_layer(cfg, er1, ec1, ev1, cfg.QB1, cfg.GB1)
    L2 = _plan_layer(cfg, er2, ec2, ev2, cfg.QB2, cfg.GB2)

    # weighted degree deg = A@1 as a [1, RPAD] partition-0 row so that
    # deg[0:1, b*128:(b+1)*128] is a [1, 128] lhsT for the rank-1 bias matmul
    deg = np.bincount(row, weights=vals, minlength=cfg.NN).astype(np.float32)
    degt = []
    for m in range(cfg.M):
        d = np.zeros((1, cfg.RPAD), dtype=np.float32)
        d[0, :cfg.RPC] = deg[m * cfg.RPC:(m + 1) * cfg.RPC]
        degt.append(np.ascontiguousarray(d.astype(ml_dtypes.bfloat16)))
    return L1, L2, degt


def build_program(cfg, L1, L2):
    nc = bacc.Bacc("TRN2", target_bir_lowering=False, debug=False,
                   num_devices=cfg.M, dynamic_dma_scratch_size=SCRATCH)

    xg_d = nc.dram_tensor("xg", [cfg.NN, cfg.IN], BF16, kind="ExternalInput")
    # wpack columns: W1a | W1b | W2 | iota | b1 | b2 (biases on row 0)
    wcols = 6 * 128
    wpack_d = nc.dram_tensor("wpack", [128, wcols], BF16, kind="ExternalInput")
    degt_d = nc.dram_tensor("degt", [1, cfg.RPAD], BF16, kind="ExternalInput")
    nsmax = max(L1["nslot"], L2["nslot"])
    ncmax = max(L1["nfcol"], L2["nfcol"])
    idx1_d = nc.dram_tensor("idx1", [128, L1["nslot"] // 16], I16,
                            kind="ExternalInput")
    idx2_d = nc.dram_tensor("idx2", [128, L2["nslot"] // 16], I16,
                            kind="ExternalInput")
    fp1_d = nc.dram_tensor("fp1", [128, 2 * L1["nfcol"]], F32,
                           kind="ExternalInput")
    fp2_d = nc.dram_tensor("fp2", [128, 2 * L2["nfcol"]], F32,
                           kind="ExternalInput")
    out_d = nc.dram_tensor("out", [cfg.RPAD, cfg.OUT], F32,
                           kind="ExternalOutput")

    r1_loc = nc.dram_tensor("r1_loc", [cfg.RPAD, cfg.HID], BF16)
    # NOTE: addr_space="Shared" would be the fast-collective path, but the
    # axon/PJRT execution backend (fake_nrt) hangs on shared scratchpads.
    r1_full = nc.dram_tensor("r1_full", [cfg.NNP, cfg.HID], BF16)

    rg = [list(range(cfg.M))]

    with tile.TileContext(nc) as tc:
        from contextlib import ExitStack
        with ExitStack() as ctx:
            const = ctx.enter_context(tc.tile_pool(name="const", bufs=1))
            idxp = ctx.enter_context(tc.tile_pool(name="idxp", bufs=1))
            fpp = ctx.enter_context(tc.tile_pool(name="fpp", bufs=1))
            gb_pool = ctx.enter_context(tc.tile_pool(name="gb", bufs=8))
            s_pool = ctx.enter_context(tc.tile_pool(name="sm", bufs=128))
            ev_pool = ctx.enter_context(tc.tile_pool(name="ev", bufs=8))
            rt_pool = ctx.enter_context(tc.tile_pool(name="rt", bufs=1))
            ot_pool = ctx.enter_context(tc.tile_pool(name="ot", bufs=2))
            psum_s = ctx.enter_context(
                tc.tile_pool(name="psum_s", bufs=6, space="PSUM"))
            psum_g = ctx.enter_context(
                tc.tile_pool(name="psum_g", bufs=2, space="PSUM"))

            # resident constants (Act-engine HWDGE so the SP queue leads
            # with the idx chunks that gate the first gather)
            wpack_sb = const.tile([128, wcols], BF16, tag="wpack", name="wp")
            nc.scalar.dma_start(wpack_sb[:], wpack_d[:, :])
            w1a = wpack_sb[:, 0:128]
            w1b = wpack_sb[:, 128:256]
            w2 = wpack_sb[:, 256:384]
            iota_sb = wpack_sb[:, 384:512]
            b1r = wpack_sb[0:1, 512:640]
            b2r = wpack_sb[0:1, 640:768]
            degt_sb = const.tile([1, cfg.RPAD], BF16, tag="degt", name="dg")
            nc.scalar.dma_start(degt_sb[:], degt_d[:, :])

            idx_sb = idxp.tile([128, nsmax // 16], I16, tag="idx", name="ix")
            fp_sb = fpp.tile([128, 2 * ncmax], F32, tag="fp", name="fp")

            def spmm_layer(plan, src_view_of_q, elem, halves, consume_block,
                           consume_group=None):
                """Edge-slot SpMM: per group: gathers, scatter matmuls into
                per-block psum (one per half).  consume_block(b, ptiles)
                evacuates psum immediately and returns a closure with the
                trailing GEMM work; closures flush after the NEXT group's
                first run so the PE never bubbles on fresh evacuations.
                """
                nfcol = plan["nfcol"]
                rloc_sb = fp_sb[:, 0:nfcol]
                sval_sb = fp_sb[:, nfcol:2 * nfcol]
                gbch = max(r["n"] for r in plan["runs"]) // 128
                ri = 0
                pending = []
                for gi, blist in enumerate(plan["groups"]):
                    ptiles = [[psum_s.tile([128, 128], F32, tag="ps",
                                           name="ps")
                               for _ in range(halves)] for _ in blist]
                    for q in range(cfg.Q):
                        run = plan["runs"][ri]
                        ri += 1
                        assert run["q"] == q
                        ioff, n = run["ioff"], run["n"]
                        gb3 = gb_pool.tile([128, gbch, elem], BF16,
                                           tag="gb", name="gb")
                        o = 0
                        while o < n:
                            nj = min(RING, n - o)
                            nc.gpsimd.dma_gather(
                                out_ap=gb3[:, o // 128:(o + nj) // 128, :],
                                in_ap=src_view_of_q(q),
                                idxs_ap=idx_sb[:, (ioff + o) // 16:
                                               (ioff + o + nj) // 16],
                                num_idxs=nj, num_idxs_reg=nj,
                                elem_size=elem,
                            )
                            o += nj
                        for c, ilist in enumerate(run["insts"]):
                            for bi, b, col, st, sp in ilist:
                                s = s_pool.tile([128, 128], BF16,
                                                tag="s", name="s")
                                nc.vector.tensor_scalar(
                                    s[:], iota_sb,
                                    rloc_sb[:, col:col + 1],
                                    sval_sb[:, col:col + 1],
                                    mybir.AluOpType.is_equal,
                                    mybir.AluOpType.mult)
                                for h in range(halves):
                                    nc.tensor.matmul(
                                        ptiles[bi][h][:, :],
                                        gb3[:, c, h * 128:(h + 1) * 128],
                                        s[:],
                                        start=st, stop=sp,
                                        skip_group_check=True)
                        if q == 0 and pending:
                            for fn in pending:
                                fn()
                            pending = []
                    pending = [consume_block(b, ptiles[bi])
                               for bi, b in enumerate(blist)]
                    if consume_group is not None:
                        pending.append(consume_group(blist))
                for fn in pending:
                    fn()

            def chunked_load(dst, src, ncols, parts=4):
                step = -(-ncols // parts)
                o = 0
                while o < ncols:
                    e = min(ncols, o + step)
                    nc.sync.dma_start(dst[:, o:e], src[:, o:e])
                    o = e

            # ================= Layer 1 =================
            chunked_load(idx_sb, idx1_d, L1["nslot"] // 16)
            chunked_load(fp_sb, fp1_d, 2 * L1["nfcol"])

            r1_sb = rt_pool.tile([128, cfg.RPAD], BF16, tag="r1", name="r1")

            def x_view(q):
                lo = q * cfg.QB1
                hi = min(cfg.NN, lo + cfg.QB1)
                return xg_d[lo:hi, :]

            def consume1(b, pts):
                # evacuate (A@X)^T halves now (Act + DVE in parallel) so the
                # psum scatter tiles free immediately; GEMM1 + rank-1 bias +
                # relu are deferred via the returned closure
                ax0 = ev_pool.tile([128, 128], BF16, tag="ax0", name="ax0")
                ax1 = ev_pool.tile([128, 128], BF16, tag="ax1", name="ax1")
                nc.scalar.copy(ax0[:], pts[0][:, :])
                nc.vector.tensor_copy(ax1[:], pts[1][:, :])

                def gemm():
                    ps = psum_g.tile([128, 128], F32, tag="g", name="g1")
                    nc.tensor.matmul(ps[:], ax0[:], w1a, start=True,
                                     stop=False, skip_group_check=True)
                    nc.tensor.matmul(ps[:], ax1[:], w1b, start=False,
                                     stop=False, skip_group_check=True)
                    nc.tensor.matmul(ps[:],
                                     degt_sb[0:1, b * 128:(b + 1) * 128],
                                     b1r, start=False, stop=True,
                                     skip_group_check=True)
                    nc.scalar.activation(r1_sb[:, b * 128:(b + 1) * 128],
                                         ps[:],
                                         mybir.ActivationFunctionType.Relu)
                return gemm

            r1_loc_r = r1_loc.rearrange("(t p) f -> p t f", p=128)
            r1_sb_r = r1_sb.rearrange("p (t f) -> p t f", f=128)

            def group1_out(blist):
                def emit():
                    b0, b1 = blist[0], blist[-1] + 1
                    nc.sync.dma_start(r1_loc_r[:, b0:b1, :],
                                      r1_sb_r[:, b0:b1, :])
                return emit

            spmm_layer(L1, x_view, cfg.IN, 2, consume1, group1_out)

            nc.gpsimd.collective_compute(
                "AllGather", mybir.AluOpType.bypass, replica_groups=rg,
                ins=[r1_loc[:, :]], outs=[r1_full[:, :]])

            # ================= Layer 2 =================
            nc.sync.dma_start(idx_sb[:, :L2["nslot"] // 16], idx2_d[:, :])
            nc.sync.dma_start(fp_sb[:, :2 * L2["nfcol"]], fp2_d[:, :])

            def r1_view(q):
                return r1_full[q * cfg.QB2:(q + 1) * cfg.QB2, :]

            out_r = out_d.rearrange("(t p) f -> p t f", p=128)

            gb2 = cfg.GB2
            ot_group = {}

            def consume2(b, pts):
                ar = ev_pool.tile([128, 128], BF16, tag="ar", name="ar",
                                  bufs=14)
                nc.scalar.copy(ar[:], pts[0][:, :])

                def gemm():
                    g0 = (b // gb2) * gb2
                    if g0 not in ot_group:
                        ot_group[g0] = ot_pool.tile(
                            [128, gb2 * 128], F32, tag="ot", name="ot")
                    ot = ot_group[g0]
                    ps = psum_g.tile([128, 128], F32, tag="g", name="g2")
                    nc.tensor.matmul(ps[:], ar[:], w2, start=True, stop=False,
                                     skip_group_check=True)
                    nc.tensor.matmul(ps[:],
                                     degt_sb[0:1, b * 128:(b + 1) * 128],
                                     b2r, start=False, stop=True,
                                     skip_group_check=True)
                    lo = (b - g0) * 128
                    nc.scalar.copy(ot[:, lo:lo + 128], ps[:])
                    return ot
                return gemm

            def group2_out(blist):
                def emit():
                    g0 = (blist[0] // gb2) * gb2
                    ot = ot_group.pop(g0)
                    nb = len(blist)
                    nc.sync.dma_start(
                        out_r[:, g0:g0 + nb, :],
                        ot[:, :nb * 128].rearrange("p (t f) -> p t f",
                                                   f=128))
                return emit

            spmm_layer(L2, r1_view, cfg.HID, 1, consume2, group2_out)

    nc.compile()
    return nc


def _prep_inputs(cfg, X, W1, b1, W2, b2, L1, L2, degt):
    bf = ml_dtypes.bfloat16
    wcols = 6 * 128
    wpack = np.zeros((128, wcols), dtype=np.float32)
    W1 = np.asarray(W1, dtype=np.float32)
    wpack[:, 0:128] = W1[0:128]
    wpack[:, 128:256] = W1[128:256]
    wpack[:, 256:384] = np.asarray(W2)
    wpack[:, 384:512] = np.arange(128, dtype=np.float32)[None, :]
    wpack[0, 512:640] = np.asarray(b1)
    wpack[0, 640:768] = np.asarray(b2)
    wpack = wpack.astype(bf)

    xg = np.ascontiguousarray(np.asarray(X, dtype=np.float32).astype(bf))
    in_maps = []
    for m in range(cfg.M):
        fp1 = np.concatenate([L1["per_core"][m]["rloc"],
                              L1["per_core"][m]["sval"]], axis=1)
        fp2 = np.concatenate([L2["per_core"][m]["rloc"],
                              L2["per_core"][m]["sval"]], axis=1)
        in_maps.append(dict(
            xg=xg, wpack=wpack, degt=degt[m],
            idx1=L1["per_core"][m]["idx"], idx2=L2["per_core"][m]["idx"],
            fp1=np.ascontiguousarray(fp1), fp2=np.ascontiguousarray(fp2)))
    return in_maps


def run(cfg, X, W1, b1, W2, b2, vals, row, col, trace=False):
    L1, L2, degt = build_plan(cfg, row, col, vals)
    nc = build_program(cfg, L1, L2)
    in_maps = _prep_inputs(cfg, X, W1, b1, W2, b2, L1, L2, degt)
    res = run_bass_kernel_spmd(nc, in_maps, list(range(cfg.M)), trace=trace)
    outs = [np.asarray(res.results[m]["out"])[:cfg.RPC] for m in range(cfg.M)]
    out = np.concatenate(outs, axis=0).astype(np.float32)
    return out, res


def kernel(X, W1, b1, W2, b2, vals, row, col):
    out, _ = run(FULL, X, W1, b1, W2, b2, vals, row, col)
    return out


# revision 45
# speedup vs baseline: 3.1754x; 3.1754x over previous
"""2-layer GCN (GEMM -> COO SpMM -> ReLU -> GEMM -> SpMM) on 8 trn2 NeuronCores.

v2 design (one collective, X-space layer 1):
  - A@(X W1 + b1) = (A@X) W1 + deg.b1^T with deg = A@1 (host bincount).
    X is a full input replicated on every core, so layer 1 needs NO
    collective: gather X rows directly (256 bf16 feats = 512B descriptors,
    full DMA efficiency), accumulate (A@X)^T per 128-row dest block on PE
    via edge-slot scatter matmuls, then a small local GEMM + rank-1 bias.
  - relu1 [RPC rows, 128] bf16 is written row-major and AllGather'd once
    (Shared-output HBM collective = NRT fast path).
  - Layer 2 = baseline SpMM structure over relu1_full: gather by (block,
    quartile), scatter-matmul to (A relu1)^T, local GEMM2 + rank-1 bias,
    row-major f32 output (no host transpose), out-writes batched per group.
  - dma_gather instructions stay <= 1024 descriptors with the default 16KB
    SWDGE scratch: larger rings/scratch sizes hang the device.
  - Block GEMMs are deferred by one group (consume closures) so psum
    evacuation never bubbles the PE; evacs split across Act and DVE.

PSUM budget (8 banks): scatter pool 6 (L1: 3 blocks x 2 halves, L2:
6 blocks x 1) + gemm pool 2.

SPMD: one program for 8 cores; per-core data varies only in input tensors.
Slot layout per (block, quartile) padded to CAP chunks of 128; pad slots
gather row 0 with val 0 (harmless).
"""

import sys

import numpy as np
import ml_dtypes

_TRN_REPO = "/opt/trn_rl_repo"
if _TRN_REPO not in sys.path:
    sys.path.insert(0, _TRN_REPO)

import concourse.bass as bass
import concourse.tile as tile
from concourse import bacc, mybir
from concourse.bass_utils import run_bass_kernel_spmd

BF16 = mybir.dt.bfloat16
F32 = mybir.dt.float32
I16 = mybir.dt.int16

RING = 1024          # SWDGE descriptor ring (scratch 16384 / 16)
SCRATCH = 16384


class Cfg:
    def __init__(self, n_nodes, in_size, hidden, out_size):
        self.M = 8
        self.NN = n_nodes
        self.IN = in_size
        self.HID = hidden
        self.OUT = out_size
        assert n_nodes % self.M == 0
        self.RPC = n_nodes // self.M          # real rows per core
        self.BL = 128
        self.NB = (self.RPC + 127) // 128
        self.RPAD = self.NB * 128             # padded rows per core
        self.NNP = self.M * self.RPAD         # padded global nodes
        self.Q = 4
        self.QB1 = (n_nodes + self.Q - 1) // self.Q      # X-space quartile
        self.QB2 = self.NNP // self.Q                    # padded-row quartile
        assert self.QB1 <= 32768 and self.QB2 <= 32768
        self.GB1 = 3                          # blocks/group, layer 1 (2 psum each)
        self.GB2 = 6                          # blocks/group, layer 2 (1 psum each)
        assert in_size % 128 == 0 and hidden == 128 and out_size == 128


FULL = Cfg(100000, 256, 128, 128)


def _plan_layer(cfg, er, ec, ev, qbase, gb):
    """Slot layout for one SpMM layer: edges (er=dest row local, ec=source
    index, ev=val) per core, bucketed by (dest block, source quartile).

    Fine-grained packing: each (b, q) segment gets a SHARED capacity =
    max-over-cores rounded up to 16 (idx alignment); segments concatenate
    within a (group, quartile) run (padded to x128 for chunk alignment), so
    chunks of 128 slots can span block boundaries.  Each (chunk, touched
    block) pair gets its own (rloc, sval) fpack column: slots outside the
    block get rloc=200 (never equal to iota 0..127) and sval=0.
    """
    M = len(er)
    NB = cfg.NB
    nq = cfg.Q
    # shared per-(b, q) capacities
    cnt = np.zeros((M, NB * nq), dtype=np.int64)
    for m in range(M):
        key = (er[m] // cfg.BL) * nq + np.minimum(ec[m] // qbase, nq - 1)
        cnt[m] = np.bincount(key, minlength=NB * nq)
    cap16 = ((cnt.max(axis=0) + 15) // 16 * 16).reshape(NB, nq)

    groups = [list(range(g, min(g + gb, NB))) for g in range(0, NB, gb)]
    slot_off = {}
    runs = []   # per (group, quartile): dict(q, ioff, n, insts)
    off = 0
    ninst = 0
    for blist in groups:
        for q in range(nq):
            ioff = off
            for b in blist:
                slot_off[(b, q)] = off
                off += int(cap16[b, q])
            n = -(-(off - ioff) // 128) * 128          # pad run to x128
            off = ioff + n
            # chunk -> touched blocks
            insts = []
            for c in range(n // 128):
                lo, hi = ioff + c * 128, ioff + (c + 1) * 128
                touched = [b for b in blist
                           if slot_off[(b, q)] < hi and
                           slot_off[(b, q)] + int(cap16[b, q]) > lo]
                ilist = []
                for b in touched:
                    ilist.append([blist.index(b), b, ninst, False, False])
                    ninst += 1
                insts.append(ilist)
            runs.append(dict(q=q, ioff=ioff, n=n, insts=insts))
    nslot = off
    nchunk = nslot // 128
    nfcol = ninst

    # start/stop flags: first/last instance per block in emission order
    first_seen = {}
    last_seen = {}
    for run in runs:
        for ilist in run["insts"]:
            for inst in ilist:
                b = inst[1]
                if b not in first_seen:
                    first_seen[b] = inst
                last_seen[b] = inst
    for b, inst in first_seen.items():
        inst[3] = True
    for b, inst in last_seen.items():
        inst[4] = True

    # shared owner map: slot -> owning block (-1 for run pad)
    owner = np.full(nslot, -1, dtype=np.int64)
    for (b, q), so in slot_off.items():
        owner[so:so + int(cap16[b, q])] = b

    per_core = []
    for m in range(M):
        erm, ecm, evm = er[m], ec[m], ev[m]
        blk = erm // cfg.BL
        order = np.lexsort((ecm, blk))
        erm, ecm, evm, blk = erm[order], ecm[order], evm[order], blk[order]

        idx16 = np.zeros(nslot, dtype=np.int16)
        rloc = np.full(nslot, 200.0, dtype=np.float32)
        sval = np.zeros(nslot, dtype=np.float32)

        bstart = np.searchsorted(blk, np.arange(NB + 1))
        for b in range(NB):
            i0, i1 = bstart[b], bstart[b + 1]
            ecb = ecm[i0:i1]
            qsplit = np.searchsorted(ecb, np.arange(nq + 1) * qbase)
            for q in range(nq):
                j0, j1 = i0 + qsplit[q], i0 + qsplit[q + 1]
                n = j1 - j0
                assert n <= cap16[b, q]
                so = slot_off[(b, q)]
                idx16[so:so + n] = (ecm[j0:j1] - q * qbase).astype(np.int16)
                rloc[so:so + n] = (erm[j0:j1] - b * cfg.BL).astype(np.float32)
                sval[so:so + n] = evm[j0:j1]

        # per-instance fpack columns [128, nfcol]
        rcols = np.full((128, nfcol), 200.0, dtype=np.float32)
        scols = np.zeros((128, nfcol), dtype=np.float32)
        for run in runs:
            for c, ilist in enumerate(run["insts"]):
                lo = run["ioff"] + c * 128
                wo = owner[lo:lo + 128]
                wr = rloc[lo:lo + 128]
                ws = sval[lo:lo + 128]
                for bi, b, col, _, _ in ilist:
                    sel = wo == b
                    rcols[:, col] = np.where(sel, wr, 200.0)
                    scols[:, col] = np.where(sel, ws, 0.0)

        idx_w = np.tile(idx16.reshape(-1, 16).T, (8, 1))
        per_core.append(dict(
            idx=np.ascontiguousarray(idx_w),
            rloc=np.ascontiguousarray(rcols),
            sval=np.ascontiguousarray(scols)))
    return dict(groups=groups, runs=runs, nslot=nslot, nchunk=nchunk,
                nfcol=nfcol, per_core=per_core)


def build_plan(cfg, row, col, vals):
    row = np.asarray(row).astype(np.int64)
    col = np.asarray(col).astype(np.int64)
    vals = np.asarray(vals).astype(np.float32)

    er1, ec1, ev1 = [], [], []
    er2, ec2, ev2 = [], [], []
    colp = (col // cfg.RPC) * cfg.RPAD + (col % cfg.RPC)  # padded-row space
    for m in range(cfg.M):
        sel = (row // cfg.RPC) == m
        er = (row[sel] - m * cfg.RPC).astype(np.int64)
        ev = vals[sel]
        er1.append(er); ec1.append(col[sel]); ev1.append(ev)
        er2.append(er.copy()); ec2.append(colp[sel]); ev2.append(ev.copy())

    L1 = _plan_layer(cfg, er1, ec1, ev1, cfg.QB1, cfg.GB1)
    L2 = _plan_layer(cfg, er2, ec2, ev2, cfg.QB2, cfg.GB2)

    # weighted degree deg = A@1 as a [1, RPAD] partition-0 row so that
    # deg[0:1, b*128:(b+1)*128] is a [1, 128] lhsT for the rank-1 bias matmul
    deg = np.bincount(row, weights=vals, minlength=cfg.NN).astype(np.float32)
    degt = []
    for m in range(cfg.M):
        d = np.zeros((1, cfg.RPAD), dtype=np.float32)
        d[0, :cfg.RPC] = deg[m * cfg.RPC:(m + 1) * cfg.RPC]
        degt.append(np.ascontiguousarray(d.astype(ml_dtypes.bfloat16)))
    return L1, L2, degt


def build_program(cfg, L1, L2):
    nc = bacc.Bacc("TRN2", target_bir_lowering=False, debug=False,
                   num_devices=cfg.M, dynamic_dma_scratch_size=SCRATCH)

    xg_d = nc.dram_tensor("xg", [cfg.NN, cfg.IN], BF16, kind="ExternalInput")
    # wpack columns: W1a | W1b | W2 | iota | b1 | b2 (biases on row 0)
    wcols = 6 * 128
    wpack_d = nc.dram_tensor("wpack", [128, wcols], BF16, kind="ExternalInput")
    degt_d = nc.dram_tensor("degt", [1, cfg.RPAD], BF16, kind="ExternalInput")
    nsmax = max(L1["nslot"], L2["nslot"])
    ncmax = max(L1["nfcol"], L2["nfcol"])
    idx1_d = nc.dram_tensor("idx1", [128, L1["nslot"] // 16], I16,
                            kind="ExternalInput")
    idx2_d = nc.dram_tensor("idx2", [128, L2["nslot"] // 16], I16,
                            kind="ExternalInput")
    fp1_d = nc.dram_tensor("fp1", [128, 2 * L1["nfcol"]], F32,
                           kind="ExternalInput")
    fp2_d = nc.dram_tensor("fp2", [128, 2 * L2["nfcol"]], F32,
                           kind="ExternalInput")
    out_d = nc.dram_tensor("out", [cfg.RPAD, cfg.OUT], F32,
                           kind="ExternalOutput")

    r1_loc = nc.dram_tensor("r1_loc", [cfg.RPAD, cfg.HID], BF16)
    # NOTE: addr_space="Shared" would be the fast-collective path, but the
    # axon/PJRT execution backend (fake_nrt) hangs on shared scratchpads.
    r1_full = nc.dram_tensor("r1_full", [cfg.NNP, cfg.HID], BF16)

    rg = [list(range(cfg.M))]

    with tile.TileContext(nc) as tc:
        from contextlib import ExitStack
        with ExitStack() as ctx:
            const = ctx.enter_context(tc.tile_pool(name="const", bufs=1))
            idxp = ctx.enter_context(tc.tile_pool(name="idxp", bufs=1))
            fpp = ctx.enter_context(tc.tile_pool(name="fpp", bufs=1))
            gb_pool = ctx.enter_context(tc.tile_pool(name="gb", bufs=8))
            s_pool = ctx.enter_context(tc.tile_pool(name="sm", bufs=128))
            ev_pool = ctx.enter_context(tc.tile_pool(name="ev", bufs=8))
            rt_pool = ctx.enter_context(tc.tile_pool(name="rt", bufs=1))
            ot_pool = ctx.enter_context(tc.tile_pool(name="ot", bufs=2))
            psum_s = ctx.enter_context(
                tc.tile_pool(name="psum_s", bufs=6, space="PSUM"))
            psum_g = ctx.enter_context(
                tc.tile_pool(name="psum_g", bufs=2, space="PSUM"))

            # resident constants (Act-engine HWDGE so the SP queue leads
            # with the idx chunks that gate the first gather)
            wpack_sb = const.tile([128, wcols], BF16, tag="wpack", name="wp")
            nc.scalar.dma_start(wpack_sb[:], wpack_d[:, :])
            w1a = wpack_sb[:, 0:128]
            w1b = wpack_sb[:, 128:256]
            w2 = wpack_sb[:, 256:384]
            iota_sb = wpack_sb[:, 384:512]
            b1r = wpack_sb[0:1, 512:640]
            b2r = wpack_sb[0:1, 640:768]
            degt_sb = const.tile([1, cfg.RPAD], BF16, tag="degt", name="dg")
            nc.scalar.dma_start(degt_sb[:], degt_d[:, :])

            idx_sb = idxp.tile([128, nsmax // 16], I16, tag="idx", name="ix")
            fp_sb = fpp.tile([128, 2 * ncmax], F32, tag="fp", name="fp")

            def spmm_layer(plan, src_view_of_q, elem, halves, consume_block,
                           consume_group=None):
                """Edge-slot SpMM: per group: gathers, scatter matmuls into
                per-block psum (one per half).  consume_block(b, ptiles)
                evacuates psum immediately and returns a closure with the
                trailing GEMM work; closures flush after the NEXT group's
                first run so the PE never bubbles on fresh evacuations.
                """
                nfcol = plan["nfcol"]
                rloc_sb = fp_sb[:, 0:nfcol]
                sval_sb = fp_sb[:, nfcol:2 * nfcol]
                gbch = max(r["n"] for r in plan["runs"]) // 128
                ri = 0
                pending = []
                for gi, blist in enumerate(plan["groups"]):
                    ptiles = [[psum_s.tile([128, 128], F32, tag="ps",
                                           name="ps")
                               for _ in range(halves)] for _ in blist]
                    for q in range(cfg.Q):
                        run = plan["runs"][ri]
                        ri += 1
                        assert run["q"] == q
                        ioff, n = run["ioff"], run["n"]
                        gb3 = gb_pool.tile([128, gbch, elem], BF16,
                                           tag="gb", name="gb")
                        o = 0
                        while o < n:
                            nj = min(RING, n - o)
                            nc.gpsimd.dma_gather(
                                out_ap=gb3[:, o // 128:(o + nj) // 128, :],
                                in_ap=src_view_of_q(q),
                                idxs_ap=idx_sb[:, (ioff + o) // 16:
                                               (ioff + o + nj) // 16],
                                num_idxs=nj, num_idxs_reg=nj,
                                elem_size=elem,
                            )
                            o += nj
                        for c, ilist in enumerate(run["insts"]):
                            for bi, b, col, st, sp in ilist:
                                s = s_pool.tile([128, 128], BF16,
                                                tag="s", name="s")
                                nc.vector.tensor_scalar(
                                    s[:], iota_sb,
                                    rloc_sb[:, col:col + 1],
                                    sval_sb[:, col:col + 1],
                                    mybir.AluOpType.is_equal,
                                    mybir.AluOpType.mult)
                                for h in range(halves):
                                    nc.tensor.matmul(
                                        ptiles[bi][h][:, :],
                                        gb3[:, c, h * 128:(h + 1) * 128],
                                        s[:],
                                        start=st, stop=sp,
                                        skip_group_check=True)
                        if q == 0 and pending:
                            for fn in pending:
                                fn()
                            pending = []
                    pending = [consume_block(b, ptiles[bi])
                               for bi, b in enumerate(blist)]
                    if consume_group is not None:
                        pending.append(consume_group(blist))
                for fn in pending:
                    fn()

            def chunked_load(dst, src, ncols, parts=4):
                step = -(-ncols // parts)
                o = 0
                while o < ncols:
                    e = min(ncols, o + step)
                    nc.sync.dma_start(dst[:, o:e], src[:, o:e])
                    o = e

            # ================= Layer 1 =================
            chunked_load(idx_sb, idx1_d, L1["nslot"] // 16)
            chunked_load(fp_sb, fp1_d, 2 * L1["nfcol"])

            r1_sb = rt_pool.tile([128, cfg.RPAD], BF16, tag="r1", name="r1")

            def x_view(q):
                lo = q * cfg.QB1
                hi = min(cfg.NN, lo + cfg.QB1)
                return xg_d[lo:hi, :]

            def consume1(b, pts):
                # evacuate (A@X)^T halves now (Act + DVE in parallel) so the
                # psum scatter tiles free immediately; GEMM1 + rank-1 bias +
                # relu are deferred via the returned closure
                ax0 = ev_pool.tile([128, 128], BF16, tag="ax0", name="ax0")
                ax1 = ev_pool.tile([128, 128], BF16, tag="ax1", name="ax1")
                nc.scalar.copy(ax0[:], pts[0][:, :])
                nc.vector.tensor_copy(ax1[:], pts[1][:, :])

                def gemm():
                    ps = psum_g.tile([128, 128], F32, tag="g", name="g1")
                    nc.tensor.matmul(ps[:], ax0[:], w1a, start=True,
                                     stop=False, skip_group_check=True)
                    nc.tensor.matmul(ps[:], ax1[:], w1b, start=False,
                                     stop=False, skip_group_check=True)
                    nc.tensor.matmul(ps[:],
                                     degt_sb[0:1, b * 128:(b + 1) * 128],
                                     b1r, start=False, stop=True,
                                     skip_group_check=True)
                    nc.scalar.activation(r1_sb[:, b * 128:(b + 1) * 128],
                                         ps[:],
                                         mybir.ActivationFunctionType.Relu)
                return gemm

            r1_loc_r = r1_loc.rearrange("(t p) f -> p t f", p=128)
            r1_sb_r = r1_sb.rearrange("p (t f) -> p t f", f=128)

            def group1_out(blist):
                def emit():
                    b0, b1 = blist[0], blist[-1] + 1
                    nc.sync.dma_start(r1_loc_r[:, b0:b1, :],
                                      r1_sb_r[:, b0:b1, :])
                return emit

            spmm_layer(L1, x_view, cfg.IN, 2, consume1, group1_out)

            nc.gpsimd.collective_compute(
                "AllGather", mybir.AluOpType.bypass, replica_groups=rg,
                ins=[r1_loc[:, :]], outs=[r1_full[:, :]])

            # ================= Layer 2 =================
            nc.sync.dma_start(idx_sb[:, :L2["nslot"] // 16], idx2_d[:, :])
            nc.sync.dma_start(fp_sb[:, :2 * L2["nfcol"]], fp2_d[:, :])

            def r1_view(q):
                return r1_full[q * cfg.QB2:(q + 1) * cfg.QB2, :]

            out_r = out_d.rearrange("(t p) f -> p t f", p=128)

            gb2 = cfg.GB2
            ot_group = {}

            def consume2(b, pts):
                ar = ev_pool.tile([128, 128], BF16, tag="ar", name="ar",
                                  bufs=14)
                nc.scalar.copy(ar[:], pts[0][:, :])

                def gemm():
                    g0 = (b // gb2) * gb2
                    if g0 not in ot_group:
                        ot_group[g0] = ot_pool.tile(
                            [128, gb2 * 128], F32, tag="ot", name="ot")
                    ot = ot_group[g0]
                    ps = psum_g.tile([128, 128], F32, tag="g", name="g2")
                    nc.tensor.matmul(ps[:], ar[:], w2, start=True, stop=False,
                                     skip_group_check=True)
                    nc.tensor.matmul(ps[:],
                                     degt_sb[0:1, b * 128:(b + 1) * 128],
                                     b2r, start=False, stop=True,
                                     skip_group_check=True)
                    lo = (b - g0) * 128
                    nc.scalar.copy(ot[:, lo:lo + 128], ps[:])
                    return ot
                return gemm

            def group2_out(blist):
                def emit():
                    g0 = (blist[0] // gb2) * gb2
                    ot = ot_group.pop(g0)
                    nb = len(blist)
                    nc.sync.dma_start(
                        out_r[:, g0:g0 + nb, :],
                        ot[:, :nb * 128].rearrange("p (t f) -> p t f",
                                                   f=128))
                return emit

            spmm_layer(L2, r1_view, cfg.HID, 1, consume2, group2_out)

    nc.compile()
    return nc


def _prep_inputs(cfg, X, W1, b1, W2, b2, L1, L2, degt):
    bf = ml_dtypes.bfloat16
    wcols = 6 * 128
    wpack = np.zeros((128, wcols), dtype=np.float32)
    W1 = np.asarray(W1, dtype=np.float32)
    wpack[:, 0:128] = W1[0:128]
    wpack[:, 128:256] = W1[128:256]
    wpack[:, 256:384] = np.asarray(W2)
    wpack[:, 384:512] = np.arange(128, dtype=np.float32)[None, :]
    wpack[0, 512:640] = np.asarray(b1)
    wpack[0, 640:768] = np.asarray(b2)
    wpack = wpack.astype(bf)

    xg = np.ascontiguousarray(np.asarray(X, dtype=np.float32).astype(bf))
    in_maps = []
    for m in range(cfg.M):
        fp1 = np.concatenate([L1["per_core"][m]["rloc"],
                              L1["per_core"][m]["sval"]], axis=1)
        fp2 = np.concatenate([L2["per_core"][m]["rloc"],
                              L2["per_core"][m]["sval"]], axis=1)
        in_maps.append(dict(
            xg=xg, wpack=wpack, degt=degt[m],
            idx1=L1["per_core"][m]["idx"], idx2=L2["per_core"][m]["idx"],
            fp1=np.ascontiguousarray(fp1), fp2=np.ascontiguousarray(fp2)))
    return in_maps


def run(cfg, X, W1, b1, W2, b2, vals, row, col, trace=False):
    L1, L2, degt = build_plan(cfg, row, col, vals)
    nc = build_program(cfg, L1, L2)
    in_maps = _prep_inputs(cfg, X, W1, b1, W2, b2, L1, L2, degt)
    res = run_bass_kernel_spmd(nc, in_maps, list(range(cfg.M)), trace=trace)
    outs = [np.asarray(res.results[m]["out"])[:cfg.RPC] for m in range(cfg.M)]
    out = np.concatenate(outs, axis=0).astype(np.float32)
    return out, res


def kernel(X, W1, b1, W2, b2, vals, row, col):
    out, _ = run(FULL, X, W1, b1, W2, b2, vals, row, col)
    return out
